# revision 1
# baseline (speedup 1.0000x reference)
"""Trainium2 Bass kernel for a dense transformer block (pre-LN, FIRE attention
bias, GELU MLP), SPMD across 8 NeuronCores with zero collectives.

Sharding: core c handles batch b=c//2 with Q-token-tile parity par=c%2
(interleaved 128-row token tiles balance the causal-attention load). K/V are
recomputed locally for the full sequence, making every sublayer token-parallel
— no collective is needed anywhere. Parity enters ONLY through input data
(xq slice + mask packing), never through addressing, so one graph runs on all
8 cores.

Per-core dataflow:
  LN1(xb) -> PE-transpose -> h1T (feature-major bf16)   [for K, V]
  LN1(xq) -> PE-transpose -> hqT                         [for Q]
  K,Q feature-major [wcol, tok]; V token-major [tok, head, hd] with a ones
  column appended per head (softmax denominator falls out of the AV matmul).
  Scores are computed TRANSPOSED [kt, qt] (lhsT=k, rhs=q); +mask via DVE on
  PSUM; exp on ACT (no max subtraction — score range is safe in f32); AV
  matmul (lhsT=P, rhs=v_aug) accumulates [qt, hd|l]; per-partition reciprocal
  normalizes. attn-proj + residual -> x2 (f32, SBUF-resident); LN2 -> h2T;
  FC+GELU -> aT; MLP-proj accumulates into x2; DMA out.
All matmuls bf16 with f32 PSUM accumulation; the residual spine stays f32.
"""
import numpy as np
import ml_dtypes

import concourse.bass as bass
import concourse.bacc as bacc
import concourse.tile as tile
from concourse import mybir
from concourse.bass_utils import run_bass_kernel_spmd
from concourse.masks import make_identity

BF16NP = ml_dtypes.bfloat16
F32 = mybir.dt.float32
BF16 = mybir.dt.bfloat16
FP8 = mybir.dt.float8e4
AF = mybir.ActivationFunctionType
P = 128
EPS = 1e-5

FULL = dict(T=2048, C=1024, H=16, F=4096)
SMALL = dict(T=512, C=512, H=8, F=2048)


def cfg_derived(cfg):
    T, C, H, F = cfg["T"], cfg["C"], cfg["H"], cfg["F"]
    d = dict(cfg)
    d["HD"] = C // H
    d["NT"] = T // P
    d["NJ"] = T // P // 2
    d["NC"] = C // P
    d["NF"] = F // P
    d["HPW"] = P // d["HD"]          # heads per 128-wide wcol chunk
    d["C5"] = min(C, 512)
    d["NH5"] = C // d["C5"]
    d["T5"] = min(T, 512)
    d["NT5"] = T // d["T5"]
    d["Q5"] = min(d["NJ"] * P, 512)
    d["NQ5"] = d["NJ"] * P // d["Q5"]
    return d


def mask_layout(d):
    """Flat element offsets of packed transposed mask blocks, per (j, h)."""
    offs = {}
    off = 0
    for j in range(d["NJ"]):
        nkc = 2 * (j + 1)
        for h in range(d["H"]):
            offs[(j, h)] = off
            off += nkc * P * P
    return offs, off


def build_graph(cfg, repeat=1, upto=99):
    d = cfg_derived(cfg)
    T, C, H, F, HD = d["T"], d["C"], d["H"], d["F"], d["HD"]
    NT, NJ, NC, NF, HPW = d["NT"], d["NJ"], d["NC"], d["NF"], d["HPW"]
    C5, NH5, T5, NT5 = d["C5"], d["NH5"], d["T5"], d["NT5"]
    Q5, NQ5 = d["Q5"], d["NQ5"]
    TQ = NJ * P                       # this core's token count
    moffs, MTOT = mask_layout(d)

    nc = bacc.Bacc("TRN2", target_bir_lowering=False, debug=False)

    xb = nc.dram_tensor("xb", [T, C], BF16, kind="ExternalInput").ap()
    xq = nc.dram_tensor("xq", [TQ, C], F32, kind="ExternalInput").ap()
    maskp = nc.dram_tensor("maskp", [MTOT], FP8, kind="ExternalInput").ap()
    wq_p = nc.dram_tensor("wq_p", [C, C], BF16, kind="ExternalInput").ap()
    wk_p = nc.dram_tensor("wk_p", [C, C], BF16, kind="ExternalInput").ap()
    wv_p = nc.dram_tensor("wv_p", [P, C // P * C], BF16, kind="ExternalInput").ap()
    wap = nc.dram_tensor("wap", [P, C // P * C], BF16, kind="ExternalInput").ap()
    wfc = nc.dram_tensor("wfc", [F, C], BF16, kind="ExternalInput").ap()
    wmp = nc.dram_tensor("wmp", [F // P, P, C], BF16, kind="ExternalInput").ap()
    bq8 = nc.dram_tensor("bq8", [C], F32, kind="ExternalInput").ap()
    bk = nc.dram_tensor("bk", [C], F32, kind="ExternalInput").ap()
    bv = nc.dram_tensor("bv", [C], F32, kind="ExternalInput").ap()
    bap = nc.dram_tensor("bap", [C], F32, kind="ExternalInput").ap()
    bfc = nc.dram_tensor("bfc", [F], F32, kind="ExternalInput").ap()
    bmp = nc.dram_tensor("bmp", [C], F32, kind="ExternalInput").ap()
    ln1g = nc.dram_tensor("ln1g", [C], F32, kind="ExternalInput").ap()
    ln1b = nc.dram_tensor("ln1b", [C], F32, kind="ExternalInput").ap()
    ln2g = nc.dram_tensor("ln2g", [C], F32, kind="ExternalInput").ap()
    ln2b = nc.dram_tensor("ln2b", [C], F32, kind="ExternalInput").ap()
    out = nc.dram_tensor("out", [TQ, C], F32, kind="ExternalOutput").ap()

    with tile.TileContext(nc) as tc:
        with tc.tile_pool(name="consts", bufs=1) as consts:

            def bcast(src1d, width, name):
                t = consts.tile([P, width], F32, name=name)
                ap = bass.AP(tensor=src1d.tensor, offset=src1d.offset,
                             ap=[[0, P], [1, width]])
                nc.sync.dma_start(out=t, in_=ap)
                return t

            def colt(src1d, nchunks, name):
                t = consts.tile([P, nchunks], F32, name=name)
                ap = bass.AP(tensor=src1d.tensor, offset=src1d.offset,
                             ap=[[1, P], [P, nchunks]])
                nc.sync.dma_start(out=t, in_=ap)
                return t

            ident = consts.tile([P, P], BF16, name="ident")
            make_identity(nc, ident)
            eps_t = consts.tile([P, 1], F32, name="eps_t")
            nc.vector.memset(eps_t, EPS)
            bv_b = bcast(bv, C, "bv_b")
            bap_b = bcast(bap, C, "bap_b")
            bmp_b = bcast(bmp, C, "bmp_b")
            bq8_c = colt(bq8, NC, "bq8_c")
            bk_c = colt(bk, NC, "bk_c")
            bfc_c = colt(bfc, NF, "bfc_c")
            g1_c = colt(ln1g, NC, "g1_c")
            b1_c = colt(ln1b, NC, "b1_c")
            g2_c = colt(ln2g, NC, "g2_c")
            b2_c = colt(ln2b, NC, "b2_c")

            with tc.tile_pool(name="psA", bufs=3, space="PSUM") as psA, \
                 tc.tile_pool(name="psB", bufs=3, space="PSUM") as psB, \
                 tc.tile_pool(name="psC", bufs=2, space="PSUM") as psC, \
                 tc.tile_pool(name="mtp", bufs=6) as mtp, \
                 tc.tile_pool(name="ptp", bufs=6) as ptp, \
                 tc.tile_pool(name="ysm", bufs=4) as ysmp:
                for rep in range(repeat):
                    _emit_iteration(upto,
                        nc, tc, d, rep,
                        ident=ident, eps_t=eps_t, bv_b=bv_b, bap_b=bap_b,
                        bmp_b=bmp_b, bq8_c=bq8_c, bk_c=bk_c, bfc_c=bfc_c,
                        g1_c=g1_c, b1_c=b1_c, g2_c=g2_c, b2_c=b2_c,
                        xb=xb, xq=xq, maskp=maskp, moffs=moffs,
                        wq_p=wq_p, wk_p=wk_p, wv_p=wv_p, wap=wap, wfc=wfc,
                        wmp=wmp, out=out,
                        psA=psA, psB=psB, psC=psC, mtp=mtp, ptp=ptp, ysmp=ysmp)
    nc.compile()
    return nc


def _emit_iteration(upto, nc, tc, d, rep, *, ident, eps_t, bv_b, bap_b, bmp_b,
                    bq8_c, bk_c, bfc_c, g1_c, b1_c, g2_c, b2_c,
                    xb, xq, maskp, moffs, wq_p, wk_p, wv_p, wap, wfc, wmp,
                    out, psA, psB, psC, mtp, ptp, ysmp):
    T, C, H, F, HD = d["T"], d["C"], d["H"], d["F"], d["HD"]
    NT, NJ, NC, NF, HPW = d["NT"], d["NJ"], d["NC"], d["NF"], d["HPW"]
    C5, NH5, T5, NT5 = d["C5"], d["NH5"], d["T5"], d["NT5"]
    Q5, NQ5 = d["Q5"], d["NQ5"]
    TQ = NJ * P
    sfx = f"_r{rep}"

    def ln_tile(lnp, xt):
        """token-major [128, C] f32 -> (x-mu)*rstd as bf16 (gain/bias applied
        post-transpose as per-partition scale/bias)."""
        ns = max(1, C // 512)
        w = C // ns
        stats = lnp.tile([P, ns, 6], F32, name="stats")
        for s in range(ns):
            nc.vector.bn_stats(out=stats[:, s, :], in_=xt[:, s * w:(s + 1) * w])
        mv = lnp.tile([P, 2], F32, name="mv")
        nc.vector.bn_aggr(out=mv, in_=stats)
        rstd = lnp.tile([P, 1], F32, name="rstd")
        nc.scalar.activation(out=rstd, in_=mv[:, 1:2], func=AF.Sqrt,
                             bias=eps_t, scale=1.0)
        nc.vector.reciprocal(out=rstd, in_=rstd)
        nmu = lnp.tile([P, 1], F32, name="nmu")
        nc.vector.tensor_mul(out=nmu, in0=mv[:, 0:1], in1=rstd)
        nc.vector.tensor_scalar_mul(out=nmu, in0=nmu, scalar1=-1.0)
        hb = lnp.tile([P, C], BF16, name="hb")
        nc.scalar.activation(out=hb, in_=xt, func=AF.Identity,
                             bias=nmu, scale=rstd)
        return hb

    # -------- persistent activations (alloc order = reverse free order) -----
    vaug, free_v = tc.tile([P, NT, H, HD + 1], BF16, name="vaug" + sfx)
    qsb, free_q = tc.tile([P, NC, TQ], BF16, name="qsb" + sfx)
    ksb, free_k = tc.tile([P, NC, T], BF16, name="ksb" + sfx)
    hqT, free_hqT = tc.tile([P, NC, TQ], BF16, name="hqT" + sfx)
    h1T, free_h1T = tc.tile([P, NC, T], BF16, name="h1T" + sfx)
    nc.vector.memset(vaug[:, :, :, HD:HD + 1], 1.0)
    # y (normalized attention output, feature-major) spills through DRAM so
    # SBUF pool lifetimes stay stack-ordered across the attention boundary
    ytd = nc.dram_tensor("ytd" + sfx, [C, TQ], BF16, kind="Internal").ap()

    # ---------------- LN1 on xq -> hqT ; LN1 on xb -> h1T ----------------
    # gain/bias application is fused into the post-transpose copy (ACT), since
    # after transposition c sits on partitions => per-partition scale/bias.
    def ln_transposed(lnp, dst, idx, g_c, b_c):
        for ci in range(NC):
            pt = psB.tile([P, P], BF16, name="sps")
            nc.tensor.transpose(pt, lnp[:, ci * P:(ci + 1) * P], ident)
            if ci % 2 == 0:
                nc.scalar.activation(
                    out=dst[:, ci, idx * P:(idx + 1) * P], in_=pt,
                    func=AF.Identity, bias=b_c[:, ci:ci + 1],
                    scale=g_c[:, ci:ci + 1])
            else:
                nc.vector.tensor_scalar(
                    out=dst[:, ci, idx * P:(idx + 1) * P], in0=pt,
                    scalar1=g_c[:, ci:ci + 1], scalar2=b_c[:, ci:ci + 1],
                    op0=mybir.AluOpType.mult, op1=mybir.AluOpType.add)

    with tc.tile_pool(name="ln1" + sfx, bufs=5) as lnp:
        for j in range(NJ):
            xt = lnp.tile([P, C], F32, name="xt")
            nc.sync.dma_start(out=xt, in_=xq[j * P:(j + 1) * P, :])
            hb = ln_tile(lnp, xt)
            ln_transposed(hb, hqT, j, g1_c, b1_c)
        for t in range(NT):
            xt = lnp.tile([P, C], BF16, name="xtb")
            nc.sync.dma_start(out=xt, in_=xb[t * P:(t + 1) * P, :])
            hb = ln_tile(lnp, xt)
            ln_transposed(hb, h1T, t, g1_c, b1_c)

    if upto <= 1:
        free_h1T(); free_hqT(); free_k(); free_q(); free_v(); return
    # ---------------- QKV projections ----------------
    with tc.tile_pool(name="wqk" + sfx, bufs=3) as wqp, \
         tc.tile_pool(name="wkA" + sfx, bufs=1) as wkap, \
         tc.tile_pool(name="wv1" + sfx, bufs=1) as wvp:
        # Q first (attention needs all of q); weights streamed
        for wq in range(NC):
            wqt = wqp.tile([P, NC, P], BF16, name="wqt")
            nc.sync.dma_start(out=wqt, in_=wq_p[wq * P:(wq + 1) * P, :]
                              .rearrange("p (ci q) -> p ci q", q=P))
            for tt in range(NQ5):
                ps = psA.tile([P, 512], F32, name="ps")[:, :Q5]
                for ci in range(NC):
                    nc.tensor.matmul(
                        ps, lhsT=wqt[:, ci, :],
                        rhs=hqT[:, ci, tt * Q5:(tt + 1) * Q5],
                        start=(ci == 0), stop=(ci == NC - 1))
                nc.vector.tensor_scalar(
                    out=qsb[:, wq, tt * Q5:(tt + 1) * Q5], in0=ps,
                    scalar1=0.125, scalar2=bq8_c[:, wq:wq + 1],
                    op0=mybir.AluOpType.mult, op1=mybir.AluOpType.add)
        # K/V weights up-front (prefetch overlaps Q pass)
        wka = wkap.tile([P, NC, NC, P], BF16, name="wka")
        for wk in range(NC):
            nc.sync.dma_start(out=wka[:, wk, :, :],
                              in_=wk_p[wk * P:(wk + 1) * P, :]
                              .rearrange("p (ci q) -> p ci q", q=P))
        wvt = wvp.tile([P, NC, C], BF16, name="wvt")
        nc.sync.dma_start(out=wvt,
                          in_=wv_p.rearrange("p (ci q) -> p ci q", q=C))
        for wk in range(NC):
            for tt in range(NT5):
                ps = psA.tile([P, 512], F32, name="ps")[:, :T5]
                for ci in range(NC):
                    nc.tensor.matmul(
                        ps, lhsT=wka[:, wk, ci, :],
                        rhs=h1T[:, ci, tt * T5:(tt + 1) * T5],
                        start=(ci == 0), stop=(ci == NC - 1))
                nc.vector.tensor_scalar(
                    out=ksb[:, wk, tt * T5:(tt + 1) * T5], in0=ps,
                    scalar1=bk_c[:, wk:wk + 1], scalar2=None,
                    op0=mybir.AluOpType.add)
        hpv = C5 // HD
        for tt in range(NT):
            pss = []
            for vh in range(NH5):
                pss.append(psA.tile([P, 512], F32, name="ps")[:, :C5])
            for ci in range(NC):
                for vh in range(NH5):
                    nc.tensor.matmul(
                        pss[vh], lhsT=h1T[:, ci, tt * P:(tt + 1) * P],
                        rhs=wvt[:, ci, vh * C5:(vh + 1) * C5],
                        start=(ci == 0), stop=(ci == NC - 1))
            for vh in range(NH5):
                nc.vector.tensor_add(
                    out=vaug[:, tt, vh * hpv:(vh + 1) * hpv, 0:HD],
                    in0=pss[vh].rearrange("p (h d) -> p h d", d=HD),
                    in1=bv_b[:, vh * C5:(vh + 1) * C5].rearrange(
                        "p (h d) -> p h d", d=HD))
    free_h1T()
    free_hqT()
    if upto <= 2:
        free_k(); free_q(); free_v(); return

    # ------- attention: head-pair software pipeline per slot -------
    # PE runs head B's score matmuls while ACT computes head A's exp, so the
    # in-order PE stream never waits on exp latency.
    _sctr = [0]

    def score_group(j, h, kc0, kw):
        hp = (h % HPW) * HD
        wk = h // HPW
        mt = mtp.tile([P, 4, P], FP8, name="mt")
        msrc = bass.AP(tensor=maskp.tensor,
                       offset=moffs[(j, h)] + kc0 * P * P,
                       ap=[[kw * P, P], [P, kw], [1, P]])
        nc.sync.dma_start(out=mt[:, :kw, :], in_=msrc)
        _sctr[0] += 1
        pool = psA if _sctr[0] % 2 else psB
        sps = pool.tile([P, 4, P], F32, name="ps" if _sctr[0] % 2 else "sps")
        for ki in range(kw):
            kc = kc0 + ki
            nc.tensor.matmul(
                sps[:, ki, :],
                lhsT=ksb[hp:hp + HD, wk, kc * P:(kc + 1) * P],
                rhs=qsb[hp:hp + HD, wk, j * P:(j + 1) * P],
                start=True, stop=True)
        nc.vector.tensor_add(out=sps[:, :kw, :], in0=sps[:, :kw, :],
                             in1=mt[:, :kw, :])
        pt = ptp.tile([P, 4, P], BF16, name="pt")
        nc.scalar.activation(out=pt[:, :kw, :], in_=sps[:, :kw, :],
                             func=AF.Exp)
        return pt

    def av_group(j, h, kc0, kw, pt, yps, off, nkc):
        for ki in range(kw):
            kc = kc0 + ki
            nc.tensor.matmul(yps[:, off:off + HD + 1], lhsT=pt[:, ki, :],
                             rhs=vaug[:, kc, h, :],
                             start=(kc == 0), stop=(kc == nkc - 1))

    def finish_head(j, h, yps, off):
        rec = ysmp.tile([P, 1], F32, name="rec")
        nc.vector.reciprocal(out=rec, in_=yps[:, off + HD:off + HD + 1])
        ynm = ysmp.tile([P, HD], BF16, name="ynm")
        nc.vector.tensor_scalar_mul(out=ynm, in0=yps[:, off:off + HD],
                                    scalar1=rec)
        ypt = psB.tile([P, P], BF16, name="sps")
        nc.tensor.transpose(ypt[:HD, :], ynm, ident)
        yts = ysmp.tile([HD, P], BF16, name="yts")
        if h % 2 == 0:
            nc.scalar.copy(out=yts, in_=ypt[:HD, :])
        else:
            nc.vector.tensor_copy(out=yts, in_=ypt[:HD, :])
        nc.sync.dma_start(
            out=ytd[h * HD:(h + 1) * HD, j * P:(j + 1) * P], in_=yts)

    # finish of pair i is deferred until after pair i+1's first score groups,
    # so the PE transpose never waits on the recip/scale chain: pair i+1's
    # score matmuls execute while DVE normalizes pair i.
    pending = [None]

    def flush_pending():
        if pending[0] is not None:
            pj, phA, phB, pyA, pyB = pending[0]
            finish_head(pj, phA, pyA, 0)
            finish_head(pj, phB, pyB, 0)
            pending[0] = None

    for j in range(NJ):
        nkc = 2 * (j + 1)
        for hA in range(0, H, 2):
            hB = hA + 1
            ypsA = psC.tile([P, HD + 1], F32, name="yps")
            ypsB = psC.tile([P, HD + 1], F32, name="yps")
            prev = None
            for gi, kc0 in enumerate(range(0, nkc, 4)):
                kw = min(4, nkc - kc0)
                ptA = score_group(j, hA, kc0, kw)
                ptB = score_group(j, hB, kc0, kw)
                if gi == 0:
                    flush_pending()
                if prev is not None:
                    pk0, pkw, pA, pB = prev
                    av_group(j, hA, pk0, pkw, pA, ypsA, 0, nkc)
                    av_group(j, hB, pk0, pkw, pB, ypsB, 0, nkc)
                prev = (kc0, kw, ptA, ptB)
            pk0, pkw, pA, pB = prev
            av_group(j, hA, pk0, pkw, pA, ypsA, 0, nkc)
            av_group(j, hB, pk0, pkw, pB, ypsB, 0, nkc)
            pending[0] = (j, hA, hB, ypsA, ypsB)
    flush_pending()
    free_k()
    free_q()
    free_v()
    if upto <= 3:
        return

    # ---------------- attn proj + residual -> x2 ----------------
    x2sb, free_x2 = tc.tile([P, NJ, C], F32, name="x2sb" + sfx)
    with tc.tile_pool(name="wapp" + sfx, bufs=1) as wapp, \
         tc.tile_pool(name="ytl" + sfx, bufs=4) as ytlp, \
         tc.tile_pool(name="xqs" + sfx, bufs=3) as xqsp:
        wapt = wapp.tile([P, NC, C], BF16, name="wapt")
        nc.sync.dma_start(out=wapt,
                          in_=wap.rearrange("p (ci q) -> p ci q", q=C))
        for j in range(NJ):
            xqt = xqsp.tile([P, C], F32, name="xqt")
            nc.sync.dma_start(out=xqt, in_=xq[j * P:(j + 1) * P, :])
            pss = []
            for nh in range(NH5):
                pss.append(psB.tile([P, 512], F32, name="sps")[:, :C5])
            for ci in range(NC):
                ytt = ytlp.tile([P, P], BF16, name="ytt")
                nc.sync.dma_start(
                    out=ytt, in_=ytd[ci * P:(ci + 1) * P, j * P:(j + 1) * P])
                for nh in range(NH5):
                    nc.tensor.matmul(
                        pss[nh], lhsT=ytt,
                        rhs=wapt[:, ci, nh * C5:(nh + 1) * C5],
                        start=(ci == 0), stop=(ci == NC - 1))
            for nh in range(NH5):
                sl = slice(nh * C5, (nh + 1) * C5)
                nc.vector.tensor_add(out=x2sb[:, j, sl], in0=pss[nh],
                                     in1=xqt[:, sl])
                nc.vector.tensor_add(out=x2sb[:, j, sl], in0=x2sb[:, j, sl],
                                     in1=bap_b[:, sl])

    if upto <= 4:
        free_x2(); return
    # ---------------- LN2 -> h2T ----------------
    h2T, free_h2T = tc.tile([P, NC, TQ], BF16, name="h2T" + sfx)
    with tc.tile_pool(name="ln2" + sfx, bufs=5) as lnp2:
        for j in range(NJ):
            hb = ln_tile(lnp2, x2sb[:, j, :])
            ln_transposed(hb, h2T, j, g2_c, b2_c)

    # mlp-proj bias folded into the x2 accumulator up front
    for j in range(NJ):
        nc.vector.tensor_add(out=x2sb[:, j, :], in0=x2sb[:, j, :], in1=bmp_b)

    # ---------------- FC+GELU -> aT, then MLP-proj accumulated into x2,
    # interleaved in groups of 8 f-chunks to bound weight residency ---------
    GRP = 8
    NG = NF // GRP
    aT, free_aT = tc.tile([P, NF, TQ], BF16, name="aT" + sfx)
    with tc.tile_pool(name="wfcp" + sfx, bufs=3) as wfcp, \
         tc.tile_pool(name="wmpp" + sfx, bufs=2) as wmpp:
        for g in range(NG):
            for wf in range(g * GRP, (g + 1) * GRP):
                wft = wfcp.tile([P, NC, P], BF16, name="wft")
                nc.sync.dma_start(out=wft, in_=wfc[wf * P:(wf + 1) * P, :]
                                  .rearrange("p (ci q) -> p ci q", q=P))
                for tt in range(NQ5):
                    ps = psA.tile([P, 512], F32, name="ps")[:, :Q5]
                    for ci in range(NC):
                        nc.tensor.matmul(
                            ps, lhsT=wft[:, ci, :],
                            rhs=h2T[:, ci, tt * Q5:(tt + 1) * Q5],
                            start=(ci == 0), stop=(ci == NC - 1))
                    nc.scalar.activation(
                        out=aT[:, wf, tt * Q5:(tt + 1) * Q5], in_=ps,
                        func=AF.Gelu_apprx_tanh, bias=bfc_c[:, wf:wf + 1],
                        scale=1.0)
            # MLP-proj for this group of f-chunks
            wmg = wmpp.tile([P, GRP, C], BF16, name="wmg")
            nc.sync.dma_start(
                out=wmg,
                in_=wmp[g * GRP:(g + 1) * GRP, :, :].rearrange(
                    "fi p q -> p fi q"))
            for j in range(NJ):
                pss = []
                for nh in range(NH5):
                    pss.append(psB.tile([P, 512], F32, name="sps")[:, :C5])
                for fi in range(GRP):
                    for nh in range(NH5):
                        nc.tensor.matmul(
                            pss[nh],
                            lhsT=aT[:, g * GRP + fi, j * P:(j + 1) * P],
                            rhs=wmg[:, fi, nh * C5:(nh + 1) * C5],
                            start=(fi == 0), stop=(fi == GRP - 1))
                for nh in range(NH5):
                    sl = slice(nh * C5, (nh + 1) * C5)
                    nc.vector.tensor_add(out=x2sb[:, j, sl],
                                         in0=x2sb[:, j, sl], in1=pss[nh])
    free_aT()
    free_h2T()

    # ---------------- write out ----------------
    for j in range(NJ):
        nc.sync.dma_start(out=out[j * P:(j + 1) * P, :], in_=x2sb[:, j, :])
    free_x2()


# ======================= host side =======================

def prep_shards(inputs, cfg):
    d = cfg_derived(cfg)
    T, C, H, F, HD = d["T"], d["C"], d["H"], d["F"], d["HD"]
    NJ, NC, NF = d["NJ"], d["NC"], d["NF"]
    moffs, MTOT = mask_layout(d)
    FP8NP = mybir.dt.np(FP8)

    x = np.ascontiguousarray(np.asarray(inputs["x"], np.float32))
    mask = np.asarray(inputs["fire_causal_mask"], np.float32)[0]  # [H,T,T]
    wqkv = np.asarray(inputs["w_qkv"], np.float32)
    bqkv = np.asarray(inputs["b_qkv"], np.float32)

    def tile_kxm(w):
        """[K, M] -> pretiled [M, K] st out[mc*P+p, ci*P+q] = w[ci*P+p, mc*P+q]
        (chunk-index transpose, intra-chunk offsets preserved), so the lhsT
        tile DMA [p, ci, q] reads fully contiguous per-partition lines."""
        Kd, M = w.shape
        w4 = w.reshape(Kd // P, P, M // P, P)
        t = w4.transpose(2, 1, 0, 3).reshape(M, Kd)
        return np.ascontiguousarray(t.astype(BF16NP))

    def tile_rhs(w):
        """[K, N] -> [P, K//P * N]: row p holds w[ci*128+p, :] ci-major."""
        Kd, N = w.shape
        t = w.reshape(Kd // P, P, N).transpose(1, 0, 2).reshape(P, -1)
        return np.ascontiguousarray(t.astype(BF16NP))

    # mask: per (j,h) groups of 4 kc, [p(kt)][kc][q] contiguous, fp8 clamped
    maskps = []
    for par in range(2):
        buf = np.empty(MTOT, FP8NP)
        for j in range(NJ):
            nkc = 2 * (j + 1)
            tq = 2 * j + par
            rows = slice(tq * P, (tq + 1) * P)
            blkT = mask[:, rows, :nkc * P]         # [H, 128q, nkc*128kt]
            for h in range(H):
                o = moffs[(j, h)]
                t = blkT[h].T.reshape(nkc, P, P)   # [kc, kt(p), q]
                for kc0 in range(0, nkc, 4):
                    kw = min(4, nkc - kc0)
                    g = t[kc0:kc0 + kw].transpose(1, 0, 2)   # [p, kc, q]
                    gq = np.clip(g, -448., 448.).astype(FP8NP).ravel()
                    buf[o + kc0 * P * P: o + (kc0 + kw) * P * P] = gq
        maskps.append(buf)

    shared = dict(
        wq_p=tile_kxm(wqkv[:, :C]),
        wk_p=tile_kxm(wqkv[:, C:2 * C]),
        wv_p=tile_rhs(wqkv[:, 2 * C:]),
        wap=tile_rhs(np.asarray(inputs["w_attn_proj"], np.float32)),
        wfc=tile_kxm(np.asarray(inputs["w_fc"], np.float32)),
        wmp=np.ascontiguousarray(
            np.asarray(inputs["w_mlp_proj"], np.float32)
            .reshape(NF, P, C).astype(BF16NP)),
        bq8=(bqkv[:C] * 0.125).astype(np.float32),
        bk=bqkv[C:2 * C].copy(), bv=bqkv[2 * C:].copy(),
        bap=np.asarray(inputs["b_attn_proj"], np.float32),
        bfc=np.asarray(inputs["b_fc"], np.float32),
        bmp=np.asarray(inputs["b_mlp_proj"], np.float32),
        ln1g=np.asarray(inputs["ln1_g"], np.float32),
        ln1b=np.asarray(inputs["ln1_b"], np.float32),
        ln2g=np.asarray(inputs["ln2_g"], np.float32),
        ln2b=np.asarray(inputs["ln2_b"], np.float32),
    )
    in_maps = []
    for c in range(8):
        b, par = c // 2, c % 2
        xq_ = np.concatenate(
            [x[b, (2 * j + par) * P:(2 * j + par + 1) * P] for j in range(NJ)], 0)
        m = dict(shared)
        m["xb"] = x[b].astype(BF16NP)
        m["xq"] = np.ascontiguousarray(xq_)
        m["maskp"] = maskps[par]
        in_maps.append(m)
    return in_maps


def assemble(results, cfg, B=4):
    d = cfg_derived(cfg)
    T, C, NJ = d["T"], d["C"], d["NJ"]
    out = np.zeros((B, T, C), np.float32)
    for c in range(8):
        b, par = c // 2, c % 2
        co = results[c]["out"]
        for j in range(NJ):
            tq = 2 * j + par
            out[b, tq * P:(tq + 1) * P] = co[j * P:(j + 1) * P]
    return out


_GRAPH_CACHE = {}


def kernel(**inputs):
    cfg = FULL
    key = "full"
    if key not in _GRAPH_CACHE:
        _GRAPH_CACHE[key] = build_graph(cfg)
    nc = _GRAPH_CACHE[key]
    in_maps = prep_shards(inputs, cfg)
    res = run_bass_kernel_spmd(nc, in_maps, core_ids=list(range(8)))
    return assemble(res.results, cfg)

